# revision 1
# baseline (speedup 1.0000x reference)
"""FALCON ObjectSomeValuesFrom forward kernel for Trainium2 (Bass/Tile).

Math: the reference computes
    c_fs[j]   = sigmoid(cw + col_j + b)
    r_fs[i,j] = sigmoid(row_i + col_j + b)
    out[i]    = max_j r_fs[i,j] * c_fs[j]
with col_j = e_j . w_r, row_i = e_i . w_l + rw, cw = c_emb . w_l,
rw = r_emb . w_l.  Both product factors are strictly increasing in col_j,
so the max over j is attained at argmax_j col_j for every i:
    out[i] = sigmoid(a_i + rw + colmax + b) * sigmoid(cw + colmax + b)
with a_i = e_i . w_l and colmax = max_j col_j.  The O(N^2) pairwise block
collapses to two GEMVs over e_all plus an elementwise sigmoid tail.

Sharding: rows are split across the 8 cores.  Each core redundantly
computes colmax over the full table (4 MB read, chunk-pipelined DMA+DVE)
and the w_l GEMV + sigmoid tail for its own 1024-row slice.  No
cross-core communication.
"""

import numpy as np

N = 8192        # 8000 named + 192 anon entities
D = 128         # emb dim
NCORES = 8
RPC = N // NCORES    # rows per core (1024)
P = 128              # SBUF partitions
NPC_FULL = N // P    # 64 rows of e_all per partition
NPC_ROWS = RPC // P  # 8 rows of the core slice per partition
CHUNK = 16           # rows-per-partition per pipeline chunk
NCHUNK = NPC_FULL // CHUNK  # 8 chunks over the full table
ACT_RED_CHUNKS = 2   # col-scan chunks whose reduce runs on ACT (rest on DVE)
GP_ROW_MUL = False   # row-pass multiply on GPSIMD, reduce on ACT
EIN_BUFS = 3
EPROD_BUFS = 3
ROW_RED_ACT = False  # row-pass reduce on ACT even when the mul stays on DVE
GP_PAIR_CHUNKS = 0   # chunks whose product gets a GPSIMD pairwise-add halving
GP_MUL_CHUNKS = 0    # trailing chunks whose multiply runs on GPSIMD
GP_MUL_FIRST = False # multiply of the (ACT-reduced) first chunk on GPSIMD
ROW_LAST = True      # trace the row pass after the col scan
CHUNK_SCHED = [16, 4, 16, 28]  # chunk sizes; ACT reduces chunks 0-1
ROW_DT_SAME = True   # row pass in the scan dtype (fp16) instead of exact f32
DMA_SPLIT = 1        # dma_starts per scan chunk (parallel queues)

_CACHE = {}
COL_DT = "fp16"  # colmax-scan precision: "f32" (exact), "fp16", or "bf16"


def _build_nc(repeat=1, col_dt="f32"):
    import concourse.bass as bass
    import concourse.bacc as bacc
    import concourse.tile as tile
    import concourse.mybir as mybir
    from concourse import bass_isa

    f32 = mybir.dt.float32
    cdt = {"f32": f32, "bf16": mybir.dt.bfloat16, "fp16": mybir.dt.float16}[col_dt]
    nc = bacc.Bacc("TRN2", target_bir_lowering=False, debug=False)

    e_full = nc.dram_tensor("e_full", [N, D], cdt, kind="ExternalInput").ap()
    rdt = cdt if (ROW_DT_SAME and col_dt != "f32") else f32
    e_rows = nc.dram_tensor("e_rows", [RPC, D], rdt, kind="ExternalInput").ap()
    wb = nc.dram_tensor("wb", [P, 2 * D], f32, kind="ExternalInput").ap()
    wr_c = nc.dram_tensor("wr_c", [P, 2 * D], cdt, kind="ExternalInput").ap()
    consts = nc.dram_tensor("consts", [P, 2], f32, kind="ExternalInput").ap()
    out = nc.dram_tensor("out", [RPC], f32, kind="ExternalOutput").ap()

    ev3 = e_full.rearrange("(p n) k -> p n k", p=P)  # [128, 64, 128]

    with tile.TileContext(nc) as tc:
        with (
            tc.tile_pool(name="ein", bufs=EIN_BUFS) as ein,
            tc.tile_pool(name="eprod", bufs=EPROD_BUFS) as eprod,
            tc.tile_pool(name="sb", bufs=1) as sb,
            tc.tile_pool(name="acc", bufs=2) as acc,
        ):
            wb_t = sb.tile([P, 2 * D], f32)
            nc.sync.dma_start(wb_t[:], wb)
            consts_t = sb.tile([P, 2], f32)
            nc.sync.dma_start(consts_t[:], consts)
            wrc_t = sb.tile([P, 2 * D], cdt)
            nc.sync.dma_start(wrc_t[:], wr_c)

            def wr_bcast(count):
                a = wrc_t[:, D : 2 * D]
                return bass.AP(a.tensor, a.offset, [a.ap[0], [0, count], a.ap[1]])

            def wl_bcast(count):
                a = wrc_t[:, 0:D] if rdt != f32 else wb_t[:, 0:D]
                return bass.AP(a.tensor, a.offset, [a.ap[0], [0, count], a.ap[1]])

            # Row slice pass: a_i = e_i . w_l for this core's rows.
            av = sb.tile([P, NPC_ROWS], f32)

            def row_pass():
                er_t = sb.tile([P, NPC_ROWS * D], rdt, name="er_t")
                er3 = er_t[:].rearrange("p (n k) -> p n k", k=D)
                erv3 = e_rows.rearrange("(p n) k -> p n k", p=P)
                nc.sync.dma_start(er3[:, :, :], erv3)
                rowp = sb.tile([P, NPC_ROWS * D], rdt, name="rowp")
                rowp3 = rowp[:].rearrange("p (n k) -> p n k", k=D)
                row_mul_eng = nc.gpsimd if GP_ROW_MUL else nc.vector
                row_mul_eng.tensor_tensor(
                    rowp3, er3, wl_bcast(NPC_ROWS), op=mybir.AluOpType.mult
                )
                if GP_ROW_MUL or ROW_RED_ACT:
                    rscratch = sb.tile([P, D], f32, name="rscratch")
                    for n in range(NPC_ROWS):
                        nc.scalar.activation(
                            rscratch[:],
                            rowp3[:, n, :],
                            mybir.ActivationFunctionType.Identity,
                            accum_out=av[:, n : n + 1],
                        )
                else:
                    nc.vector.reduce_sum(av[:], rowp3, axis=mybir.AxisListType.X)

            if not ROW_LAST:
                row_pass()

            # Full-table scan (repeat times for benchmarking; repeat=1 in
            # production): chunk-pipelined DMA -> mul -> per-chunk reduce.
            sched = CHUNK_SCHED or [CHUNK] * NCHUNK
            assert sum(sched) == NPC_FULL
            ncnk = len(sched)
            starts = [sum(sched[:i]) for i in range(ncnk)]
            colm_run = None
            for r in range(repeat):
                colv = acc.tile([P, NPC_FULL], f32, tag="colv")
                for c in range(ncnk):
                    cs, cn = starts[c], sched[c]
                    et = ein.tile([P, cn * D], cdt, tag=f"echunk{c}")
                    et3 = et[:].rearrange("p (n k) -> p n k", k=D)
                    dsp = DMA_SPLIT if cn % DMA_SPLIT == 0 else 1
                    dstep = cn // dsp
                    for d in range(dsp):
                        nc.sync.dma_start(
                            et3[:, d * dstep : (d + 1) * dstep, :],
                            ev3[:, cs + d * dstep : cs + (d + 1) * dstep, :],
                        )
                    pt = eprod.tile([P, cn * D], cdt, tag=f"pchunk{c}")
                    pt3 = pt[:].rearrange("p (n k) -> p n k", k=D)
                    mul_eng = (
                        nc.gpsimd
                        if (c >= ncnk - GP_MUL_CHUNKS or (GP_MUL_FIRST and c == 0))
                        else nc.vector
                    )
                    mul_eng.tensor_tensor(
                        pt3, et3, wr_bcast(cn), op=mybir.AluOpType.mult
                    )
                    if c < ACT_RED_CHUNKS:
                        ascr = eprod.tile([P, D], cdt, tag="ascratch")
                        for n in range(cn):
                            nc.scalar.activation(
                                ascr[:],
                                pt3[:, n, :],
                                mybir.ActivationFunctionType.Identity,
                                accum_out=colv[:, cs + n : cs + n + 1],
                            )
                    elif c < ACT_RED_CHUNKS + GP_PAIR_CHUNKS:
                        hp = eprod.tile([P, cn * D // 2], f32, tag="hchunk")
                        hp3 = hp[:].rearrange("p (n k) -> p n k", k=D // 2)
                        nc.gpsimd.tensor_tensor(
                            hp3,
                            pt3[:, :, 0 : D // 2],
                            pt3[:, :, D // 2 : D],
                            op=mybir.AluOpType.add,
                        )
                        nc.vector.reduce_sum(
                            colv[:, cs : cs + cn], hp3, axis=mybir.AxisListType.X
                        )
                    else:
                        nc.vector.reduce_sum(
                            colv[:, cs : cs + cn], pt3, axis=mybir.AxisListType.X
                        )
                colm = acc.tile([P, 1], f32, tag="colm")
                nc.vector.reduce_max(colm[:], colv[:], axis=mybir.AxisListType.X)
                if colm_run is None:
                    colm_run = colm
                else:
                    prev = colm_run
                    colm_run = acc.tile([P, 1], f32, tag="colmrun")
                    nc.vector.tensor_tensor(
                        colm_run[:], prev[:], colm[:], op=mybir.AluOpType.max
                    )

            if ROW_LAST:
                row_pass()

            colmax = sb.tile([P, 1], f32)
            nc.gpsimd.partition_all_reduce(
                colmax[:], colm_run[:], channels=P, reduce_op=bass_isa.ReduceOp.max
            )

            # K1 = colmax + (rw + b);  K2 = sigmoid(colmax + (cw + b))
            k1 = sb.tile([P, 1], f32)
            nc.vector.tensor_tensor(
                k1[:], colmax[:], consts_t[:, 0:1], op=mybir.AluOpType.add
            )
            k2p = sb.tile([P, 1], f32)
            nc.vector.tensor_tensor(
                k2p[:], colmax[:], consts_t[:, 1:2], op=mybir.AluOpType.add
            )
            k2 = sb.tile([P, 1], f32)
            nc.scalar.activation(k2[:], k2p[:], mybir.ActivationFunctionType.Sigmoid)

            # out = sigmoid(a + K1) * K2
            so = sb.tile([P, NPC_ROWS], f32)
            nc.scalar.activation(
                so[:],
                av[:],
                mybir.ActivationFunctionType.Sigmoid,
                bias=k1[:, 0:1],
            )
            fo = sb.tile([P, NPC_ROWS], f32)
            nc.vector.tensor_scalar_mul(fo[:], so[:], k2[:, 0:1])

            outv = out.rearrange("(p n) -> p n", p=P)
            nc.sync.dma_start(outv, fo[:])

    nc.compile()
    return nc


def get_nc(repeat=1, col_dt="f32"):
    key = ("nc", repeat, col_dt)
    if key not in _CACHE:
        _CACHE[key] = _build_nc(repeat, col_dt)
    return _CACHE[key]


def prepare_in_maps(
    anon_e_emb, e_table, c_table, r_table, fc0_w, fc0_b, c_id, r_id, col_dt="f32"
):
    import ml_dtypes
    e_all = np.ascontiguousarray(
        np.concatenate(
            [np.asarray(e_table, np.float32), np.asarray(anon_e_emb, np.float32)], 0
        )
    )
    fc0_w = np.asarray(fc0_w, np.float32)
    w_l = fc0_w[0, :D]
    b = np.float32(np.asarray(fc0_b, np.float32)[0])
    c_emb = np.asarray(c_table, np.float32)[int(c_id)]
    r_emb = np.asarray(r_table, np.float32)[int(r_id)]
    rw = np.float32(np.dot(r_emb, w_l))
    cw = np.float32(np.dot(c_emb, w_l))

    wb = np.ascontiguousarray(np.broadcast_to(fc0_w[0], (P, 2 * D))).astype(np.float32)
    consts = np.empty((P, 2), np.float32)
    consts[:, 0] = rw + b
    consts[:, 1] = cw + b

    if col_dt == "f32":
        e_col = e_all
        wr_col = wb
        e_row_arr = e_all
    else:
        ndt = ml_dtypes.bfloat16 if col_dt == "bf16" else np.float16
        e_col = np.ascontiguousarray(e_all.astype(ndt))
        wr_col = np.ascontiguousarray(wb.astype(ndt))
        e_row_arr = e_col if ROW_DT_SAME else e_all

    in_maps = []
    for c in range(NCORES):
        in_maps.append(
            {
                "e_full": e_col,
                "wr_c": wr_col,
                "e_rows": np.ascontiguousarray(e_row_arr[c * RPC : (c + 1) * RPC]),
                "wb": wb,
                "consts": consts,
            }
        )
    return in_maps


def run(inputs, trace=False, trace_kwargs=None, repeat=1, col_dt=COL_DT):
    from concourse.bass_utils import run_bass_kernel_spmd

    nc = get_nc(repeat, col_dt)
    in_maps = prepare_in_maps(**inputs, col_dt=col_dt)
    res = run_bass_kernel_spmd(
        nc,
        in_maps,
        core_ids=list(range(NCORES)),
        trace=trace,
        **(trace_kwargs or {}),
    )
    out = np.concatenate([res.results[c]["out"] for c in range(NCORES)])
    return out, res


def kernel(**inputs) -> np.ndarray:
    out, _ = run(inputs, trace=False)
    return out



# revision 2
# speedup vs baseline: 2.2047x; 2.2047x over previous
"""FALCON ObjectSomeValuesFrom forward kernel for Trainium2 (Bass/Tile).

Math: the reference computes
    c_fs[j]   = sigmoid(cw + col_j + b)
    r_fs[i,j] = sigmoid(row_i + col_j + b)
    out[i]    = max_j r_fs[i,j] * c_fs[j]
with col_j = e_j . w_r, row_i = e_i . w_l + rw, cw = c_emb . w_l,
rw = r_emb . w_l.  Both product factors are strictly increasing in col_j,
so the max over j is attained at argmax_j col_j for every i:
    out[i] = sigmoid(a_i + rw + colmax + b) * sigmoid(cw + colmax + b)
with a_i = e_i . w_l and colmax = max_j col_j.  The O(N^2) pairwise block
collapses to two GEMVs over e_all plus an elementwise sigmoid tail.

Implementation: the table is pre-transposed on the host to eT [128, 8192]
(feature dim on partitions) and quantized to fp8-e3m4 with a power-of-two
scale.  Both GEMVs then run on the tensor engine as 64 self-loading
matmuls (stationary = eT 128x128 block, moving = [w_l, w_r] as 2 fp8
columns), which fuses the multiply and the d-reduction and leaves the
vector engine free.  A strided reduce_max + gpsimd partition_all_reduce
produce colmax broadcast to all partitions, and the activation engine
computes the sigmoid tail.  Every core runs the identical program on the
identical full inputs (the scan is the dominant cost and is inherently
global -- colmax needs every row -- and the modeled collective cost is
far larger than replicating it), so core 0's output is the full answer.
"""

import numpy as np

N = 8192        # 8000 named + 192 anon entities
D = 128         # emb dim
NCORES = 8
RPC = N // NCORES    # kept for test.py compatibility
P = 128              # SBUF partitions
NBLK = N // P        # 64 matmul blocks of 128 rows
DMA_CHUNKS = 4
SE = 4.0             # host scale on e before fp8 quantization
SW = 4.0             # host scale on w before fp8 quantization
COL_DT = "fp8"       # table precision: "fp8" (e3m4) or "fp16"

_CACHE = {}


def _build_nc(repeat=1, col_dt=COL_DT):
    import concourse.bass as bass
    import concourse.bacc as bacc
    import concourse.tile as tile
    import concourse.mybir as mybir
    from concourse import bass_isa

    f32 = mybir.dt.float32
    tdt = {"fp8": mybir.dt.float8e3, "fp16": mybir.dt.float16}[col_dt]
    inv_s = (1.0 / (SE * SW)) if col_dt == "fp8" else 1.0

    nc = bacc.Bacc("TRN2", target_bir_lowering=False, debug=False)

    eTd = nc.dram_tensor("eT", [P, N], tdt, kind="ExternalInput").ap()
    wcd = nc.dram_tensor("wc", [P, 2], tdt, kind="ExternalInput").ap()
    constsd = nc.dram_tensor("consts", [P, 2], f32, kind="ExternalInput").ap()
    out = nc.dram_tensor("out", [N], f32, kind="ExternalOutput").ap()

    with tile.TileContext(nc) as tc:
        with (
            tc.tile_pool(name="sb", bufs=1) as sb,
            tc.tile_pool(name="ps", bufs=1, space=bass.MemorySpace.PSUM) as ps,
        ):
            wc_t = sb.tile([P, 2], tdt)
            nc.gpsimd.dma_start(wc_t[:], wcd)
            consts_t = sb.tile([P, 2], f32)
            nc.gpsimd.dma_start(consts_t[:], constsd)

            et = sb.tile([P, N], tdt)
            step = N // DMA_CHUNKS
            for c in range(DMA_CHUNKS):
                nc.sync.dma_start(
                    et[:, c * step : (c + 1) * step],
                    eTd[:, c * step : (c + 1) * step],
                )

            # pt[p, 2b + t]: t=0 -> a_{128b+p} (w_l GEMV), t=1 -> col_{128b+p}
            pt = ps.tile([P, 2 * NBLK], f32)
            pt3 = pt[:].rearrange("p (n two) -> p n two", two=2)
            for b in range(NBLK):
                nc.tensor.matmul(
                    pt3[:, b, :],
                    et[:, b * P : (b + 1) * P],
                    wc_t[:, 0:2],
                    start=True,
                    stop=True,
                )

            # Strided views over the interleaved PSUM columns.
            a_row = pt[:, 0:1]
            rowv = bass.AP(a_row.tensor, a_row.offset, [a_row.ap[0], [2, NBLK]])
            a_col = pt[:, 1:2]
            colv = bass.AP(a_col.tensor, a_col.offset, [a_col.ap[0], [2, NBLK]])

            colm = sb.tile([P, 1], f32)
            nc.vector.reduce_max(colm[:], colv, axis=mybir.AxisListType.X)
            colmax = sb.tile([P, 1], f32)
            nc.gpsimd.partition_all_reduce(
                colmax[:], colm[:], channels=P, reduce_op=bass_isa.ReduceOp.max
            )

            # k1 = colmax/s + (rw + b);  k2 = sigmoid(colmax/s + (cw + b))
            k1 = sb.tile([P, 1], f32)
            nc.vector.tensor_scalar(
                k1[:], colmax[:], inv_s, consts_t[:, 0:1],
                op0=mybir.AluOpType.mult, op1=mybir.AluOpType.add,
            )
            k2 = sb.tile([P, 1], f32)
            nc.scalar.activation(
                k2[:], colmax[:], mybir.ActivationFunctionType.Sigmoid,
                bias=consts_t[:, 1:2], scale=inv_s,
            )

            # out = sigmoid(a/s + k1) * k2
            so = sb.tile([P, NBLK], f32)
            nc.scalar.activation(
                so[:], rowv, mybir.ActivationFunctionType.Sigmoid,
                bias=k1[:, 0:1], scale=inv_s,
            )
            fo = sb.tile([P, NBLK], f32)
            nc.vector.tensor_scalar_mul(fo[:], so[:], k2[:, 0:1])

            # out_dev[p*64 + n] = true_out[n*128 + p]; host un-interleaves.
            outv = out.rearrange("(p n) -> p n", p=P)
            nc.sync.dma_start(outv, fo[:])

    nc.compile()
    return nc


def get_nc(repeat=1, col_dt=COL_DT):
    key = ("nc", repeat, col_dt)
    if key not in _CACHE:
        _CACHE[key] = _build_nc(repeat, col_dt)
    return _CACHE[key]


def prepare_in_maps(
    anon_e_emb, e_table, c_table, r_table, fc0_w, fc0_b, c_id, r_id, col_dt=COL_DT
):
    import ml_dtypes

    e_all = np.concatenate(
        [np.asarray(e_table, np.float32), np.asarray(anon_e_emb, np.float32)], 0
    )
    fc0_w = np.asarray(fc0_w, np.float32)
    w_l = fc0_w[0, :D]
    w_r = fc0_w[0, D:]
    b = np.float32(np.asarray(fc0_b, np.float32)[0])
    c_emb = np.asarray(c_table, np.float32)[int(c_id)]
    r_emb = np.asarray(r_table, np.float32)[int(r_id)]
    rw = np.float32(np.dot(r_emb, w_l))
    cw = np.float32(np.dot(c_emb, w_l))

    if col_dt == "fp8":
        ndt, se, sw = ml_dtypes.float8_e3m4, SE, SW
    else:
        ndt, se, sw = np.float16, 1.0, 1.0
    eT = np.ascontiguousarray((e_all.T * se).astype(ndt))          # [128, 8192]
    wc = np.ascontiguousarray(
        (np.stack([w_l, w_r], axis=1) * sw).astype(ndt)            # [128, 2]
    )
    consts = np.empty((P, 2), np.float32)
    consts[:, 0] = rw + b
    consts[:, 1] = cw + b

    in_map = {"eT": eT, "wc": wc, "consts": consts}
    return [dict(in_map) for _ in range(NCORES)]


def unscramble(out_dev: np.ndarray) -> np.ndarray:
    """Device layout [p*NBLK + n] -> true row order [n*P + p]."""
    return np.ascontiguousarray(out_dev.reshape(P, NBLK).T.reshape(-1))


def run(inputs, trace=False, trace_kwargs=None, repeat=1, col_dt=COL_DT):
    from concourse.bass_utils import run_bass_kernel_spmd

    nc = get_nc(repeat, col_dt)
    in_maps = prepare_in_maps(**inputs, col_dt=col_dt)
    res = run_bass_kernel_spmd(
        nc,
        in_maps,
        core_ids=list(range(NCORES)),
        trace=trace,
        **(trace_kwargs or {}),
    )
    out = unscramble(np.asarray(res.results[0]["out"]))
    return out, res


def kernel(**inputs) -> np.ndarray:
    out, _ = run(inputs, trace=False)
    return out


# revision 8
# speedup vs baseline: 2.2182x; 1.0061x over previous
"""FALCON ObjectSomeValuesFrom forward kernel for Trainium2 (Bass/Tile).

Math: the reference computes
    c_fs[j]   = sigmoid(cw + col_j + b)
    r_fs[i,j] = sigmoid(row_i + col_j + b)
    out[i]    = max_j r_fs[i,j] * c_fs[j]
with col_j = e_j . w_r, row_i = e_i . w_l + rw, cw = c_emb . w_l,
rw = r_emb . w_l.  Both product factors are strictly increasing in col_j,
so the max over j is attained at argmax_j col_j for every i:
    out[i] = sigmoid(a_i + rw + colmax + b) * sigmoid(cw + colmax + b)
with a_i = e_i . w_l and colmax = max_j col_j.  The O(N^2) pairwise block
collapses to two GEMVs over e_all plus an elementwise sigmoid tail.

Implementation: the table is pre-transposed on the host to eT [128, 8192]
(feature dim on partitions) and quantized to fp8-e3m4 with a power-of-two
scale.  Both GEMVs then run on the tensor engine as 64 self-loading
matmuls (stationary = eT 128x128 block, moving = [w_l, w_r] as 2 fp8
columns), which fuses the multiply and the d-reduction and leaves the
vector engine free.  Per-DMA-chunk strided reduce_max passes + a gpsimd
partition_all_reduce produce colmax broadcast to all partitions, the
activation engine computes the sigmoid tail, and the result is written
back with a pre-prepared SWDGE kv_writeback fired by trigger_dma (which
skips the descriptor-generation latency of a plain DMA on the critical
path).  Every core runs the identical program on the identical full
inputs (colmax needs every row, and the modeled collective cost is far
larger than replicating the scan), so core 0's output is the full answer.
"""

import numpy as np

N = 8192        # 8000 named + 192 anon entities
D = 128         # emb dim
NCORES = 8
RPC = N // NCORES    # kept for test.py compatibility
P = 128              # SBUF partitions
NBLK = N // P        # 64 matmul blocks of 128 rows
DMA_CHUNKS = 4
BPC = NBLK // DMA_CHUNKS   # matmul blocks per DMA chunk
SE = 4.0             # host scale on e before fp8 quantization
SW = 4.0             # host scale on w before fp8 quantization
COL_DT = "fp8"       # table precision: "fp8" (e3m4) or "fp16"

_CACHE = {}


def _build_nc(repeat=1, col_dt=COL_DT):
    import concourse.bass as bass
    import concourse.bacc as bacc
    import concourse.tile as tile
    import concourse.mybir as mybir
    from concourse import bass_isa

    f32 = mybir.dt.float32
    i32 = mybir.dt.int32
    u8 = mybir.dt.uint8
    tdt = {"fp8": mybir.dt.float8e3, "fp16": mybir.dt.float16}[col_dt]
    inv_s = (1.0 / (SE * SW)) if col_dt == "fp8" else 1.0

    nc = bacc.Bacc("TRN2", target_bir_lowering=False, debug=False)

    eTd = nc.dram_tensor("eT", [P, N], tdt, kind="ExternalInput").ap()
    # aux: bytes 0-1 = [w_l, w_r] in table dtype (fp8), 4-11 = consts f32
    auxd = nc.dram_tensor("aux", [P, 12], u8, kind="ExternalInput").ap()
    out = nc.dram_tensor("out", [N], f32, kind="ExternalOutput").ap()

    with tile.TileContext(nc) as tc:
        with (
            tc.tile_pool(name="sb", bufs=1) as sb,
            tc.tile_pool(name="ps", bufs=1, space=bass.MemorySpace.PSUM) as ps,
        ):
            aux_t = sb.tile([P, 12], u8)
            nc.gpsimd.dma_start(aux_t[:], auxd)
            wc_t = aux_t[:, 0:4].bitcast(tdt)       # [P, 4/tdt-size], cols 0:2 used
            consts_t = aux_t[:, 4:12].bitcast(f32)  # [P, 2]

            # Dummy sigmoid so the activation table load is scheduled early,
            # overlapping the table DMA instead of sitting on the tail.
            scr = sb.tile([P, 1], f32)
            nc.vector.memset(scr[:], 0.0)
            scr2 = sb.tile([P, 1], f32)
            nc.scalar.activation(scr2[:], scr[:], mybir.ActivationFunctionType.Sigmoid)

            # Writeback indices for the prepared kv_writeback (all zeros).
            idxs = sb.tile([P, 1], i32)
            nc.vector.memset(idxs[:], 0)

            et = sb.tile([P, N], tdt)
            step = N // DMA_CHUNKS
            for c in range(DMA_CHUNKS):
                nc.sync.dma_start(
                    et[:, c * step : (c + 1) * step],
                    eTd[:, c * step : (c + 1) * step],
                )

            # pt[p, 2b + t]: t=0 -> a_{128b+p} (w_l GEMV), t=1 -> col_{128b+p}
            pt = ps.tile([P, 2 * NBLK], f32)
            pt3 = pt[:].rearrange("p (n two) -> p n two", two=2)
            cm = sb.tile([P, DMA_CHUNKS], f32)
            a_col = pt[:, 1:2]
            for c in range(DMA_CHUNKS):
                for b in range(c * BPC, (c + 1) * BPC):
                    nc.tensor.matmul(
                        pt3[:, b, :],
                        et[:, b * P : (b + 1) * P],
                        wc_t[:, 0:2],
                        start=True,
                        stop=True,
                    )
                # Partial max over this chunk's col entries (overlaps later DMA)
                colv_c = bass.AP(
                    a_col.tensor, a_col.offset + 2 * c * BPC, [a_col.ap[0], [2, BPC]]
                )
                nc.vector.reduce_max(cm[:, c : c + 1], colv_c, axis=mybir.AxisListType.X)

            colm = sb.tile([P, 1], f32)
            nc.vector.reduce_max(colm[:], cm[:], axis=mybir.AxisListType.X)
            colmax = sb.tile([P, 1], f32)
            nc.gpsimd.partition_all_reduce(
                colmax[:], colm[:], channels=P, reduce_op=bass_isa.ReduceOp.max
            )

            # k1 = colmax/s + (rw + b);  k2 = sigmoid(colmax/s + (cw + b))
            k1 = sb.tile([P, 1], f32)
            nc.vector.tensor_scalar(
                k1[:], colmax[:], inv_s, consts_t[:, 0:1],
                op0=mybir.AluOpType.mult, op1=mybir.AluOpType.add,
            )
            k2 = sb.tile([P, 1], f32)
            nc.scalar.activation(
                k2[:], colmax[:], mybir.ActivationFunctionType.Sigmoid,
                bias=consts_t[:, 1:2], scale=inv_s,
            )

            # out = sigmoid(a/s + k1) * k2
            a_row = pt[:, 0:1]
            rowv = bass.AP(a_row.tensor, a_row.offset, [a_row.ap[0], [2, NBLK]])
            so = sb.tile([P, NBLK], f32)
            nc.scalar.activation(
                so[:], rowv, mybir.ActivationFunctionType.Sigmoid,
                bias=k1[:, 0:1], scale=inv_s,
            )
            fo = sb.tile([P, NBLK], f32)
            nc.vector.tensor_scalar_mul(fo[:], so[:], k2[:, 0:1])

            # Prepared SWDGE writeback: out_dev[p*64 + n] = fo[p, n]; the
            # trigger inherits the data dependency on fo, so only
            # trigger+transfer+sem sit on the tail (no HWDGE/DGE latency).
            out4 = out.rearrange("(b dhi dho n) -> b dhi dho n", b=1, dhi=P, dho=1)
            fo4 = fo[:].rearrange("p (dho b n) -> p dho b n", dho=1, b=1)
            wb_sem = nc.alloc_semaphore("wb_dma")
            nc.gpsimd.kv_writeback(
                out4, fo4, idxs[:], prepare_only=True, sem=wb_sem
            )
            nc.gpsimd.trigger_dma(count=None)
            nc.gpsimd.wait_ge(wb_sem, 16)

    # Tile schedules the prep on a DMASW proc lane and makes the final drain
    # wait on that lane's semaphore, but no instruction in the stream ever
    # increments it (the exec paths handle it via internal lane bookkeeping;
    # the no_exec TimelineSim does not).  Attach the lane increment to the
    # explicit wait_ge(wb_sem) instruction: it only fires after the actual
    # DMA-completion semaphore, so ordering and timing stay honest on both
    # the simulators and hardware.
    fn = nc.m.functions[0]
    insts = [i for blk in fn.blocks for i in blk.instructions]
    updated = set()
    waited = {}
    wb_sem_id = wb_sem.num if hasattr(wb_sem, "num") else None
    wb_waiter = None
    for inst in insts:
        si = inst.sync_info
        if si is None:
            continue
        for u in si.on_update or []:
            updated.add(u.id)
        for w in si.on_wait or []:
            if (w.ant_name or "").startswith("DMASW"):
                waited[w.id] = (w.ant_name, w.wait_value)
            if w.ant_name == "wb_dma" and type(inst).__name__ == "InstEventSemaphore":
                wb_waiter = inst
    starved = {i: v for i, v in waited.items() if i not in updated}
    if starved:
        assert wb_waiter is not None, "wb_dma waiter not found for DMASW patch"
        si = wb_waiter.sync_info
        fixes = [
            mybir.SyncUpdate(
                sync_type="semaphore", id=sid, ant_name=name,
                update_mode="sem-add-imm", update_value=val, update_reg=None,
            )
            for sid, (name, val) in starved.items()
        ]
        si.on_update = fixes + list(si.on_update or [])

    nc.compile()
    return nc


def get_nc(repeat=1, col_dt=COL_DT):
    key = ("nc", repeat, col_dt)
    if key not in _CACHE:
        _CACHE[key] = _build_nc(repeat, col_dt)
    return _CACHE[key]


def prepare_in_maps(
    anon_e_emb, e_table, c_table, r_table, fc0_w, fc0_b, c_id, r_id, col_dt=COL_DT
):
    import ml_dtypes

    e_all = np.concatenate(
        [np.asarray(e_table, np.float32), np.asarray(anon_e_emb, np.float32)], 0
    )
    fc0_w = np.asarray(fc0_w, np.float32)
    w_l = fc0_w[0, :D]
    w_r = fc0_w[0, D:]
    b = np.float32(np.asarray(fc0_b, np.float32)[0])
    c_emb = np.asarray(c_table, np.float32)[int(c_id)]
    r_emb = np.asarray(r_table, np.float32)[int(r_id)]
    rw = np.float32(np.dot(r_emb, w_l))
    cw = np.float32(np.dot(c_emb, w_l))

    if col_dt == "fp8":
        ndt, se, sw = ml_dtypes.float8_e3m4, SE, SW
    else:
        ndt, se, sw = np.float16, 1.0, 1.0
    eT = np.ascontiguousarray((e_all.T * se).astype(ndt))          # [128, 8192]
    wc = np.ascontiguousarray(
        (np.stack([w_l, w_r], axis=1) * sw).astype(ndt)            # [128, 2]
    )
    aux = np.zeros((P, 12), np.uint8)
    aux[:, 0:2] = wc.view(np.uint8)
    consts = np.empty((P, 2), np.float32)
    consts[:, 0] = rw + b
    consts[:, 1] = cw + b
    aux[:, 4:12] = consts.view(np.uint8)

    in_map = {"eT": eT, "aux": aux}
    return [dict(in_map) for _ in range(NCORES)]


def unscramble(out_dev: np.ndarray) -> np.ndarray:
    """Device layout [p*NBLK + n] -> true row order [n*P + p]."""
    return np.ascontiguousarray(out_dev.reshape(P, NBLK).T.reshape(-1))


def run(inputs, trace=False, trace_kwargs=None, repeat=1, col_dt=COL_DT):
    from concourse.bass_utils import run_bass_kernel_spmd

    nc = get_nc(repeat, col_dt)
    in_maps = prepare_in_maps(**inputs, col_dt=col_dt)
    res = run_bass_kernel_spmd(
        nc,
        in_maps,
        core_ids=list(range(NCORES)),
        trace=trace,
        **(trace_kwargs or {}),
    )
    out = unscramble(np.asarray(res.results[0]["out"]))
    return out, res


def kernel(**inputs) -> np.ndarray:
    out, _ = run(inputs, trace=False)
    return out
